# revision 22
# baseline (speedup 1.0000x reference)
"""Trainium2 Bass kernel for nn_BRGEHHNet (gnn_message_passing).

Contract: kernel(**inputs) takes FULL unsharded inputs (as produced by
setup_inputs) and returns the FULL (25, 2048) float32 output.

Strategy: data-parallel over the batch dim across 8 NeuronCores.
Each core handles a 256-column batch shard; the small anova/adjacency
and per-agent critic weights are replicated. BatchNorm statistics are
over the full batch, so every core loads the full transposed states
and computes the stats locally (no collectives - collective latency
floor of ~7us would dominate this kernel).

v2 over the 95.6us baseline:
  - all weights + states shipped host-side as bf16 (halves HBM traffic
    to ~11 MB/core; rel-err budget 2e-2 tolerates it)
  - DMA issue spread across sync/scalar/gpsimd queues so no queue
    serializes the stream
  - stats pipelined per 128-feature tile as sT tiles land
  - attention psum lives in two dedicated banks (no bank reuse ->
    att matmuls never wait on the vector folds)
  - mask prep moved to gpsimd; vector does stats + folds only

Math notes:
  - att_out = emb * all_att broadcast followed by per-agent critics is
    folded into the critic weights: h1 = emb @ (all_att.T expanded * w1T).
  - The adjacency scatter (last-write-wins, matching jax-CPU/torch
    semantics) is encoded host-side as a 0/1 selection matrix S_sel
    (150 x 3200, uint8); neighbor_att = S_sel[:, r].T @ bi_att on the PE.
  - w2/w3 per-agent critics become block-diagonal matmuls; biases are
    folded into the PSUM accumulation as K=1 matmuls against a ones row.
  - The action gather is a one-hot mask multiply on the vector engine.
"""

import os
import numpy as np
import ml_dtypes

import concourse.bacc as bacc
import concourse.mybir as mybir
import concourse.tile as tile
from concourse import bass_utils

N_CORES = 8
A = 25          # agents
B = 2048        # batch
S = 32          # state dim
F = A * S       # 800 features (contraction of M1)
E = 3200        # EHH_HID (= 25 * 128)
R = A * 12      # 300 critic hidden rows
INTER = 150
NA = 4
BSH = B // N_CORES  # 256 per-core batch shard

F_T = [128] * 6 + [32]          # feature tiles (800 = 6*128 + 32)
E_MT = E // 128                  # 25 output tiles of M1
R_SPLIT = [(0, 128), (128, 256), (256, 300)]   # (a,k) row tiling
WCH = 1024                       # ehh_w column-chunk width (8 mt per chunk)

DT = mybir.dt
F32 = DT.float32
BF16 = DT.bfloat16
I32 = DT.int32
U8 = DT.uint8

TRACE = os.environ.get("BASS_KERNEL_TRACE", "0") == "1"
LAST_EXEC_NS = None

_CACHE = {}

BF = ml_dtypes.bfloat16


def _build_program():
    nc = bacc.Bacc("TRN2", target_bir_lowering=False, debug=False,
                   num_devices=N_CORES)

    sT_d = nc.dram_tensor("sT", [F, B], BF16, kind="ExternalInput")
    ehh_w_d = nc.dram_tensor("ehh_w", [F, E], BF16, kind="ExternalInput")
    # w1R / aselfR are host-rearranged into the exact SBUF layout
    # (partition-major), so their DMAs are single simple 2D patterns.
    w1R_d = nc.dram_tensor("w1R", [128, E_MT * R], BF16, kind="ExternalInput")
    ssel_d = nc.dram_tensor("ssel", [INTER, E], U8, kind="ExternalInput")
    aselfR_d = nc.dram_tensor("aselfR", [128, E_MT * A], BF16,
                              kind="ExternalInput")
    bi_d = nc.dram_tensor("bi", [INTER, A], BF16, kind="ExternalInput")
    bd2_d = nc.dram_tensor("bd2", [R, R], BF16, kind="ExternalInput")
    bd3_d = nc.dram_tensor("bd3", [R, 128], BF16, kind="ExternalInput")
    b1_d = nc.dram_tensor("b1r", [1, R], BF16, kind="ExternalInput")
    b2_d = nc.dram_tensor("b2r", [1, R], BF16, kind="ExternalInput")
    b3_d = nc.dram_tensor("b3r", [1, 128], BF16, kind="ExternalInput")
    ones_d = nc.dram_tensor("ones", [1, BSH], BF16, kind="ExternalInput")
    act_d = nc.dram_tensor("act", [A, BSH], I32, kind="ExternalInput")
    out_d = nc.dram_tensor("out", [A, BSH], F32, kind="ExternalOutput")

    with tile.TileContext(nc) as tc:
        with (
            tc.tile_pool(name="const", bufs=1) as cpool,
            tc.tile_pool(name="xt", bufs=7) as xt_pool,
            tc.tile_pool(name="xn", bufs=7) as xn_pool,
            tc.tile_pool(name="st", bufs=7) as st_pool,
            tc.tile_pool(name="big", bufs=1) as big_pool,
            tc.tile_pool(name="w1e", bufs=25) as w1e_pool,
            tc.tile_pool(name="emb", bufs=25) as emb_pool,
            tc.tile_pool(name="hh", bufs=8) as h_pool,
            tc.tile_pool(name="ps", bufs=3, space="PSUM") as ps_pool,
            tc.tile_pool(name="psatt", bufs=2, space="PSUM") as psatt_pool,
            tc.tile_pool(name="psh1", bufs=3, space="PSUM") as psh1_pool,
        ):
            # ================= DMA issue =================
            # sync queue: the sT stream first (stats critical path; queue
            # order gives it priority over the big weight chunks behind it),
            # then the ehh_w column-chunks, then late-need small tensors.
            xt = []
            for k in range(7):
                rows = F_T[k]
                t = xt_pool.tile([128, B], BF16, tag="xt", name=f"xt_{k}")
                nc.sync.dma_start(t[0:rows, :],
                                  sT_d.ap()[k * 128:k * 128 + rows, :])
                xt.append(t)
            act_i = cpool.tile([A, BSH], I32, tag="acti")
            nc.sync.dma_start(act_i[:], act_d.ap())
            bd2_t = []
            for j, (c0, c1) in enumerate(R_SPLIT):
                t = cpool.tile([c1 - c0, R], BF16, tag=f"bd2_{j}",
                               name=f"bd2t_{j}")
                nc.sync.dma_start(t[:], bd2_d.ap()[c0:c1, :])
                bd2_t.append(t)
            bd3_t = []
            for j, (c0, c1) in enumerate(R_SPLIT):
                t = cpool.tile([c1 - c0, 128], BF16, tag=f"bd3_{j}",
                               name=f"bd3t_{j}")
                nc.sync.dma_start(t[:], bd3_d.ap()[c0:c1, :])
                bd3_t.append(t)

            # gpsimd queue: ssel (casting DMA: gpsimd-only), then the small
            # attention tables and the w1R slices (all simple 2D patterns).
            ssel0 = cpool.tile([128, E], BF16, tag="ssel0")
            ssel1 = cpool.tile([INTER - 128, E], BF16, tag="ssel1")
            nc.gpsimd.dma_start(ssel0[:], ssel_d.ap()[0:128, :])
            nc.gpsimd.dma_start(ssel1[:], ssel_d.ap()[128:INTER, :])
            aself_t = cpool.tile([128, E_MT * A], BF16, tag="aselfR")
            nc.gpsimd.dma_start(aself_t[:], aselfR_d.ap())
            bi0 = cpool.tile([128, A], BF16, tag="bi0")
            bi1 = cpool.tile([INTER - 128, A], BF16, tag="bi1")
            nc.gpsimd.dma_start(bi0[:], bi_d.ap()[0:128, :])
            nc.gpsimd.dma_start(bi1[:], bi_d.ap()[128:INTER, :])
            # PRIORITY GATE: the sT stream owns the HBM until the stats
            # inputs have landed — hold the big weight streams back by making
            # the gpsimd queue wait on the second-to-last sT tile.
            gate_scr = cpool.tile([1, 64], BF16, tag="gate")
            nc.gpsimd.tensor_copy(gate_scr[0:1, :], xt[5][0:1, 0:64])
            # gpsimd also carries the big weight streams, column-chunked so
            # mm1/mm2 can start after the first chunk; w1R slices interleave
            wfull = big_pool.tile([128, 7 * E], BF16, tag="wfull")
            w1full = big_pool.tile([128, E_MT * R], BF16, tag="w1full")
            NCH = 5
            CW = E // NCH      # 640-column chunks (5 mt each)
            for g in range(NCH):
                g0 = g * CW
                g1 = g0 + CW
                nc.gpsimd.dma_start(
                    wfull[:].rearrange("p (k c) -> p k c", c=E)[:, 0:6, g0:g1],
                    ehh_w_d.ap()[0:768, g0:g1]
                    .rearrange("(k p) c -> p k c", p=128))
                nc.gpsimd.dma_start(
                    wfull[0:32, 6 * E + g0:6 * E + g1],
                    ehh_w_d.ap()[768:800, g0:g1])
                nc.gpsimd.dma_start(
                    w1full[:, g * 5 * R:(g + 1) * 5 * R],
                    w1R_d.ap()[:, g * 5 * R:(g + 1) * 5 * R])

            # scalar queue: tiny rows (simple, fast issues), then sqrt +
            # leaky work arrives behind them.
            ones_t = cpool.tile([1, BSH], BF16, tag="ones")
            nc.scalar.dma_start(ones_t[:], ones_d.ap())
            b1_t = cpool.tile([1, R], BF16, tag="b1")
            b2_t = cpool.tile([1, R], BF16, tag="b2")
            b3_t = cpool.tile([1, 128], BF16, tag="b3")
            nc.scalar.dma_start(b1_t[:], b1_d.ap())
            nc.scalar.dma_start(b2_t[:], b2_d.ap())
            nc.scalar.dma_start(b3_t[:], b3_d.ap())

            # ============ attention matmuls (tensor, runs first) ============
            # two dedicated psum banks, no reuse -> no wait on vector folds
            psatt_a = psatt_pool.tile([128, 15 * A], F32, tag="psatt",
                                      name="psatt_a")
            psatt_b = psatt_pool.tile([128, 10 * A], F32, tag="psatt",
                                      name="psatt_b")

            def att_sl(mt):
                if mt < 15:
                    return psatt_a[:, mt * A:(mt + 1) * A]
                return psatt_b[:, (mt - 15) * A:(mt - 14) * A]

            for mt in range(E_MT):
                sl = att_sl(mt)
                nc.tensor.matmul(sl, ssel0[:, mt * 128:(mt + 1) * 128],
                                 bi0[:], start=True, stop=False)
                nc.tensor.matmul(sl, ssel1[:, mt * 128:(mt + 1) * 128],
                                 bi1[:], start=False, stop=True)

            # ============ stats on vector, pipelined per k-tile ============
            # bn_stats runs ~96G elem/s and makes vector the gate of the
            # whole main loop; plain square + tensor_reduce(X) keeps up
            # with the sT arrival rate instead.
            sq_scr = cpool.tile([128, B], BF16, tag="sqscr")
            xn = []
            for k in range(7):
                rows = F_T[k]
                ssum = st_pool.tile([128, 8], F32, tag="st", name=f"ssum_{k}")
                # [0]=sum [1]=sumsq [2]=mean [3]=ex2+eps [4]=mean^2
                # [5]=var+eps [6]=sig [7]=rsig
                nc.vector.tensor_tensor(
                    out=sq_scr[0:rows, :], in0=xt[k][0:rows, :],
                    in1=xt[k][0:rows, :], op=mybir.AluOpType.mult)
                nc.vector.tensor_reduce(
                    ssum[0:rows, 0:1], xt[k][0:rows, :],
                    axis=mybir.AxisListType.X, op=mybir.AluOpType.add)
                nc.vector.tensor_reduce(
                    ssum[0:rows, 1:2], sq_scr[0:rows, :],
                    axis=mybir.AxisListType.X, op=mybir.AluOpType.add)
                nc.vector.tensor_scalar(
                    ssum[0:rows, 2:3], ssum[0:rows, 0:1], 1.0 / B, None,
                    op0=mybir.AluOpType.mult)
                nc.vector.tensor_scalar(
                    ssum[0:rows, 3:4], ssum[0:rows, 1:2], 1.0 / B, 1e-5,
                    op0=mybir.AluOpType.mult, op1=mybir.AluOpType.add)
                nc.vector.tensor_tensor(
                    out=ssum[0:rows, 4:5], in0=ssum[0:rows, 2:3],
                    in1=ssum[0:rows, 2:3], op=mybir.AluOpType.mult)
                nc.vector.tensor_tensor(
                    out=ssum[0:rows, 5:6], in0=ssum[0:rows, 3:4],
                    in1=ssum[0:rows, 4:5], op=mybir.AluOpType.subtract)
                nc.scalar.activation(
                    ssum[0:rows, 6:7], ssum[0:rows, 5:6],
                    mybir.ActivationFunctionType.Sqrt)
                nc.vector.reciprocal(ssum[0:rows, 7:8], ssum[0:rows, 6:7])
                xnk = xn_pool.tile([128, BSH], BF16, tag="xn", name=f"xn_{k}")
                nc.vector.tensor_scalar(
                    xnk[0:rows, :], xt[k][0:rows, 0:BSH],
                    ssum[0:rows, 2:3], ssum[0:rows, 7:8],
                    op0=mybir.AluOpType.subtract, op1=mybir.AluOpType.mult)
                xn.append(xnk)

            # ============ spn add + W1eff fold (vector) ============
            w1e_all = []
            for mt in range(E_MT):
                sl = att_sl(mt)
                spn_sl = aself_t[:, mt * A:(mt + 1) * A]
                nc.vector.tensor_tensor(out=spn_sl, in0=spn_sl, in1=sl,
                                        op=mybir.AluOpType.add)
                w1t = w1full[:, mt * R:(mt + 1) * R]
                w1e_t = w1e_pool.tile([128, R], BF16, tag="w1e",
                                      name=f"w1e_{mt}")
                nc.vector.tensor_tensor(
                    out=w1e_t[:].rearrange("p (a k) -> p a k", k=12),
                    in0=w1t.rearrange("p (a k) -> p a k", k=12),
                    in1=spn_sl.unsqueeze(2).broadcast_to((128, A, 12)),
                    op=mybir.AluOpType.mult)
                w1e_all.append(w1e_t)

            # ============ mask prep (gpsimd, after its DMA issues) ============
            act_f = cpool.tile([A, BSH], F32, tag="actf")
            nc.gpsimd.tensor_copy(act_f[:], act_i[:])
            masks = []
            for c4 in range(NA):
                mask = cpool.tile([A, BSH], F32, tag=f"mask_{c4}",
                                  name=f"mask_{c4}")
                nc.gpsimd.tensor_scalar(
                    mask[:], act_f[:], float(c4), None,
                    op0=mybir.AluOpType.is_equal)
                masks.append(mask)

            # ============ main loop: M1 + pipelined M2 ============
            h1ps = [psh1_pool.tile([128, BSH], F32, tag="h1ps",
                                   name=f"h1ps_{j}") for j in range(3)]
            embs = []

            def emit_m2(mt):
                for j, (c0, c1) in enumerate(R_SPLIT):
                    nc.tensor.matmul(h1ps[j][0:c1 - c0, :],
                                     w1e_all[mt][:, c0:c1], embs[mt][:],
                                     start=(mt == 0), stop=False)

            for mt in range(E_MT):
                ps_mt = ps_pool.tile([128, BSH], F32, tag="ps",
                                     name=f"psm_{mt}")
                for k in range(7):
                    rows = F_T[k]
                    lhsT = wfull[0:rows,
                                 k * E + mt * 128:k * E + (mt + 1) * 128]
                    nc.tensor.matmul(ps_mt[:], lhsT, xn[k][0:rows, :],
                                     start=(k == 0), stop=(k == 6))
                emb = emb_pool.tile([128, BSH], BF16, tag="emb",
                                    name=f"emb_{mt}")
                nc.scalar.activation(emb[:], ps_mt[:],
                                     mybir.ActivationFunctionType.Lrelu,
                                     alpha=0.01)
                embs.append(emb)
            # M2 runs AFTER the whole M1 stream: long uninterrupted matmul
            # runs let the PE p-state ramp to full clock (all 25 emb tiles
            # fit in SBUF), and M2 never stalls the queue on a vector fold.
            for mt in range(E_MT):
                emit_m2(mt)

            # finish M2: bias row, then leaky
            h1 = []
            for j, (c0, c1) in enumerate(R_SPLIT):
                w = c1 - c0
                nc.tensor.matmul(h1ps[j][0:w, :], b1_t[:, c0:c1], ones_t[:],
                                 start=False, stop=True)
                t = h_pool.tile([128, BSH], BF16, tag=f"h1_{j}",
                                name=f"h1_{j}")
                nc.scalar.activation(t[0:w, :], h1ps[j][0:w, :],
                                     mybir.ActivationFunctionType.Lrelu,
                                     alpha=0.01)
                h1.append(t)

            # M3: h2 = leaky(BD2^T @ h1 + b2)
            h2 = []
            for j, (c0, c1) in enumerate(R_SPLIT):
                w = c1 - c0
                ps3 = ps_pool.tile([128, BSH], F32, tag="ps", name=f"ps3_{j}")
                for k3, (k0, k1) in enumerate(R_SPLIT):
                    nc.tensor.matmul(ps3[0:w, :], bd2_t[k3][:, c0:c1],
                                     h1[k3][0:k1 - k0, :],
                                     start=(k3 == 0), stop=False)
                nc.tensor.matmul(ps3[0:w, :], b2_t[:, c0:c1], ones_t[:],
                                 start=False, stop=True)
                t = h_pool.tile([128, BSH], BF16, tag=f"h2_{j}",
                                name=f"h2_{j}")
                nc.scalar.activation(t[0:w, :], ps3[0:w, :],
                                     mybir.ActivationFunctionType.Lrelu,
                                     alpha=0.01)
                h2.append(t)

            # M4: all_q^T (rows = c*32+a) = BD3^T @ h2 + b3
            ps_q = ps_pool.tile([128, BSH], F32, tag="ps", name="psq")
            for k4, (k0, k1) in enumerate(R_SPLIT):
                nc.tensor.matmul(ps_q[:], bd3_t[k4][:, :],
                                 h2[k4][0:k1 - k0, :],
                                 start=(k4 == 0), stop=False)
            nc.tensor.matmul(ps_q[:], b3_t[:], ones_t[:], start=False,
                             stop=True)

            # gather: q[a, b] = all_q[c(a,b)*32+a, b] via one-hot masks
            qs = []
            for c4 in range(NA):
                qc = cpool.tile([A, BSH], F32, tag=f"qc_{c4}",
                                name=f"qc_{c4}")
                nc.vector.tensor_tensor(
                    out=qc[:], in0=ps_q[c4 * 32:c4 * 32 + A, :],
                    in1=masks[c4][:], op=mybir.AluOpType.mult)
                qs.append(qc)
            nc.vector.tensor_tensor(out=qs[0][:], in0=qs[0][:], in1=qs[1][:],
                                    op=mybir.AluOpType.add)
            nc.vector.tensor_tensor(out=qs[2][:], in0=qs[2][:], in1=qs[3][:],
                                    op=mybir.AluOpType.add)
            nc.vector.tensor_tensor(out=qs[0][:], in0=qs[0][:], in1=qs[2][:],
                                    op=mybir.AluOpType.add)
            nc.sync.dma_start(out_d.ap(), qs[0][:])

    nc.compile()
    return nc


def _host_prep(inputs):
    states = np.asarray(inputs["states"], dtype=np.float32)
    ehh_w = np.asarray(inputs["ehh_w"], dtype=np.float32)
    anova = np.asarray(inputs["anova"], dtype=np.float32)
    w1 = np.asarray(inputs["w1"], dtype=np.float32)
    b1 = np.asarray(inputs["b1"], dtype=np.float32)
    w2 = np.asarray(inputs["w2"], dtype=np.float32)
    b2 = np.asarray(inputs["b2"], dtype=np.float32)
    w3 = np.asarray(inputs["w3"], dtype=np.float32)
    b3 = np.asarray(inputs["b3"], dtype=np.float32)
    actions = np.asarray(inputs["actions"], dtype=np.int32)
    adj = np.asarray(inputs["adj"], dtype=np.int64)

    sT = np.ascontiguousarray(
        states.transpose(0, 2, 1).reshape(F, B)).astype(BF)
    w1T = w1.transpose(1, 0, 2).reshape(E, R)
    # partition-major rearranges (exact SBUF layout -> simple 2D DMAs)
    w1R = np.ascontiguousarray(
        w1T.reshape(E_MT, 128, R).transpose(1, 0, 2).reshape(128, E_MT * R)
    ).astype(BF)
    aselfR = np.ascontiguousarray(
        anova[:E].reshape(E_MT, 128, A).transpose(1, 0, 2)
        .reshape(128, E_MT * A)).astype(BF)

    # adjacency scatter -> winning source row per target (last write wins,
    # col-3 scatter applied after col-1 scatter)
    src = np.full(E, -1, dtype=np.int64)
    for e in range(adj.shape[0]):
        src[adj[e, 1]] = adj[e, 0]
    for e in range(adj.shape[0]):
        src[adj[e, 3]] = adj[e, 0]
    ssel = np.zeros((INTER, E), dtype=np.uint8)
    hit = np.nonzero(src >= 0)[0]
    ssel[src[hit], hit] = 1

    bd2 = np.zeros((R, R), dtype=np.float32)
    bd3 = np.zeros((R, 128), dtype=np.float32)
    b3r = np.zeros((1, 128), dtype=np.float32)
    for a in range(A):
        bd2[12 * a:12 * a + 12, 12 * a:12 * a + 12] = w2[a]
        for c in range(NA):
            bd3[12 * a:12 * a + 12, c * 32 + a] = w3[a, :, c]
            b3r[0, c * 32 + a] = b3[a, c]

    common = {
        "ehh_w": np.ascontiguousarray(ehh_w).astype(BF),
        "w1R": w1R,
        "ssel": ssel,
        "aselfR": aselfR,
        "bi": np.ascontiguousarray(anova[E:]).astype(BF),
        "bd2": bd2.astype(BF),
        "bd3": bd3.astype(BF),
        "b1r": b1.reshape(1, R).astype(BF),
        "b2r": b2.reshape(1, R).astype(BF),
        "b3r": b3r.astype(BF),
        "ones": np.ones((1, BSH), dtype=BF),
    }
    in_maps = []
    for c in range(N_CORES):
        m = dict(common)
        m["sT"] = np.ascontiguousarray(np.roll(sT, -BSH * c, axis=1))
        m["act"] = np.ascontiguousarray(actions[:, BSH * c:BSH * (c + 1)])
        in_maps.append(m)
    return in_maps


def kernel(**inputs):
    global LAST_EXEC_NS
    if "nc" not in _CACHE:
        _CACHE["nc"] = _build_program()
    nc = _CACHE["nc"]
    in_maps = _host_prep(inputs)
    kwargs = {}
    if TRACE:
        kwargs["trace"] = True
    res = bass_utils.run_bass_kernel_spmd(
        nc, in_maps, core_ids=list(range(N_CORES)), **kwargs)
    LAST_EXEC_NS = res.exec_time_ns
    q = np.empty((A, B), dtype=np.float32)
    for c in range(N_CORES):
        q[:, BSH * c:BSH * (c + 1)] = res.results[c]["out"]
    return q


# revision 24
# speedup vs baseline: 1.2331x; 1.2331x over previous
"""Trainium2 Bass kernel for nn_BRGEHHNet (gnn_message_passing).

Contract: kernel(**inputs) takes FULL unsharded inputs (as produced by
setup_inputs) and returns the FULL (25, 2048) float32 output.

Strategy: data-parallel over the batch dim across 8 NeuronCores.
Each core handles a 256-column batch shard; the small anova/adjacency
and per-agent critic weights are replicated. BatchNorm statistics are
over the full batch, so every core loads the full transposed states
and computes the stats locally (no collectives - collective latency
floor of ~7us would dominate this kernel).

v2 over the 95.6us baseline:
  - all weights + states shipped host-side as bf16 (halves HBM traffic
    to ~11 MB/core; rel-err budget 2e-2 tolerates it)
  - DMA issue spread across sync/scalar/gpsimd queues so no queue
    serializes the stream
  - stats pipelined per 128-feature tile as sT tiles land
  - attention psum lives in two dedicated banks (no bank reuse ->
    att matmuls never wait on the vector folds)
  - mask prep moved to gpsimd; vector does stats + folds only

Math notes:
  - att_out = emb * all_att broadcast followed by per-agent critics is
    folded into the critic weights: h1 = emb @ (all_att.T expanded * w1T).
  - The adjacency scatter (last-write-wins, matching jax-CPU/torch
    semantics) is encoded host-side as a 0/1 selection matrix S_sel
    (150 x 3200, uint8); neighbor_att = S_sel[:, r].T @ bi_att on the PE.
  - w2/w3 per-agent critics become block-diagonal matmuls; biases are
    folded into the PSUM accumulation as K=1 matmuls against a ones row.
  - The action gather is a one-hot mask multiply on the vector engine.
"""

import os
import numpy as np
import ml_dtypes

import concourse.bacc as bacc
import concourse.mybir as mybir
import concourse.tile as tile
from concourse import bass_utils

N_CORES = 8
A = 25          # agents
B = 2048        # batch
S = 32          # state dim
F = A * S       # 800 features (contraction of M1)
E = 3200        # EHH_HID (= 25 * 128)
R = A * 12      # 300 critic hidden rows
INTER = 150
NA = 4
BSH = B // N_CORES  # 256 per-core batch shard

F_T = [128] * 6 + [32]          # feature tiles (800 = 6*128 + 32)
E_MT = E // 128                  # 25 output tiles of M1
R_SPLIT = [(0, 128), (128, 256), (256, 300)]   # (a,k) row tiling
WCH = 1024                       # ehh_w column-chunk width (8 mt per chunk)

DT = mybir.dt
F32 = DT.float32
BF16 = DT.bfloat16
I32 = DT.int32
U8 = DT.uint8

TRACE = os.environ.get("BASS_KERNEL_TRACE", "0") == "1"
LAST_EXEC_NS = None

_CACHE = {}

BF = ml_dtypes.bfloat16


def _build_program():
    nc = bacc.Bacc("TRN2", target_bir_lowering=False, debug=False,
                   num_devices=N_CORES)

    sT_d = nc.dram_tensor("sT", [F, B], BF16, kind="ExternalInput")
    ehh_w_d = nc.dram_tensor("ehh_w", [F, E], BF16, kind="ExternalInput")
    # w1R / aselfR are host-rearranged into the exact SBUF layout
    # (partition-major), so their DMAs are single simple 2D patterns.
    w1R_d = nc.dram_tensor("w1R", [128, E_MT * R], BF16, kind="ExternalInput")
    ssel_d = nc.dram_tensor("ssel", [INTER, E], U8, kind="ExternalInput")
    aselfR_d = nc.dram_tensor("aselfR", [128, E_MT * A], BF16,
                              kind="ExternalInput")
    bi_d = nc.dram_tensor("bi", [INTER, A], BF16, kind="ExternalInput")
    bd2_d = nc.dram_tensor("bd2", [R, R], BF16, kind="ExternalInput")
    bd3_d = nc.dram_tensor("bd3", [R, 128], BF16, kind="ExternalInput")
    b1_d = nc.dram_tensor("b1r", [1, R], BF16, kind="ExternalInput")
    b2_d = nc.dram_tensor("b2r", [1, R], BF16, kind="ExternalInput")
    b3_d = nc.dram_tensor("b3r", [1, 128], BF16, kind="ExternalInput")
    ones_d = nc.dram_tensor("ones", [1, BSH], BF16, kind="ExternalInput")
    act_d = nc.dram_tensor("act", [A, BSH], I32, kind="ExternalInput")
    out_d = nc.dram_tensor("out", [A, BSH], F32, kind="ExternalOutput")

    with tile.TileContext(nc) as tc:
        with (
            tc.tile_pool(name="const", bufs=1) as cpool,
            tc.tile_pool(name="xt", bufs=7) as xt_pool,
            tc.tile_pool(name="xn", bufs=7) as xn_pool,
            tc.tile_pool(name="st", bufs=7) as st_pool,
            tc.tile_pool(name="big", bufs=1) as big_pool,
            tc.tile_pool(name="w1e", bufs=25) as w1e_pool,
            tc.tile_pool(name="emb", bufs=12) as emb_pool,
            tc.tile_pool(name="hh", bufs=8) as h_pool,
            tc.tile_pool(name="ps", bufs=3, space="PSUM") as ps_pool,
            tc.tile_pool(name="psatt", bufs=2, space="PSUM") as psatt_pool,
            tc.tile_pool(name="psh1", bufs=3, space="PSUM") as psh1_pool,
        ):
            # ================= DMA issue =================
            # sync queue: the sT stream first (stats critical path; queue
            # order gives it priority over the big weight chunks behind it),
            # then the ehh_w column-chunks, then late-need small tensors.
            xt = []
            for k in range(7):
                rows = F_T[k]
                t = xt_pool.tile([128, B], BF16, tag="xt", name=f"xt_{k}")
                nc.sync.dma_start(t[0:rows, :],
                                  sT_d.ap()[k * 128:k * 128 + rows, :])
                xt.append(t)
            act_i = cpool.tile([A, BSH], I32, tag="acti")
            nc.sync.dma_start(act_i[:], act_d.ap())
            bd2_t = []
            for j, (c0, c1) in enumerate(R_SPLIT):
                t = cpool.tile([c1 - c0, R], BF16, tag=f"bd2_{j}",
                               name=f"bd2t_{j}")
                nc.sync.dma_start(t[:], bd2_d.ap()[c0:c1, :])
                bd2_t.append(t)
            bd3_t = []
            for j, (c0, c1) in enumerate(R_SPLIT):
                t = cpool.tile([c1 - c0, 128], BF16, tag=f"bd3_{j}",
                               name=f"bd3t_{j}")
                nc.sync.dma_start(t[:], bd3_d.ap()[c0:c1, :])
                bd3_t.append(t)

            # gpsimd queue: ssel (casting DMA: gpsimd-only), then the small
            # attention tables and the w1R slices (all simple 2D patterns).
            ssel0 = cpool.tile([128, E], BF16, tag="ssel0")
            ssel1 = cpool.tile([INTER - 128, E], BF16, tag="ssel1")
            nc.gpsimd.dma_start(ssel0[:], ssel_d.ap()[0:128, :])
            nc.gpsimd.dma_start(ssel1[:], ssel_d.ap()[128:INTER, :])
            aself_t = cpool.tile([128, E_MT * A], BF16, tag="aselfR")
            nc.gpsimd.dma_start(aself_t[:], aselfR_d.ap())
            bi0 = cpool.tile([128, A], BF16, tag="bi0")
            bi1 = cpool.tile([INTER - 128, A], BF16, tag="bi1")
            nc.gpsimd.dma_start(bi0[:], bi_d.ap()[0:128, :])
            nc.gpsimd.dma_start(bi1[:], bi_d.ap()[128:INTER, :])
            # PRIORITY GATE: the sT stream owns the HBM until the stats
            # inputs have landed — hold the big weight streams back by making
            # the gpsimd queue wait on the second-to-last sT tile.
            gate_scr = cpool.tile([1, 64], BF16, tag="gate")
            nc.gpsimd.tensor_copy(gate_scr[0:1, :], xt[5][0:1, 0:64])
            # gpsimd also carries the big weight streams, column-chunked so
            # mm1/mm2 can start after the first chunk; w1R slices interleave
            wfull = big_pool.tile([128, 7 * E], BF16, tag="wfull")
            w1full = big_pool.tile([128, E_MT * R], BF16, tag="w1full")
            NCH = 5
            CW = E // NCH      # 640-column chunks (5 mt each)
            for g in range(NCH):
                g0 = g * CW
                g1 = g0 + CW
                nc.gpsimd.dma_start(
                    wfull[:].rearrange("p (k c) -> p k c", c=E)[:, 0:6, g0:g1],
                    ehh_w_d.ap()[0:768, g0:g1]
                    .rearrange("(k p) c -> p k c", p=128))
                nc.gpsimd.dma_start(
                    wfull[0:32, 6 * E + g0:6 * E + g1],
                    ehh_w_d.ap()[768:800, g0:g1])
                nc.gpsimd.dma_start(
                    w1full[:, g * 5 * R:(g + 1) * 5 * R],
                    w1R_d.ap()[:, g * 5 * R:(g + 1) * 5 * R])

            # scalar queue: tiny rows (simple, fast issues), then sqrt +
            # leaky work arrives behind them.
            ones_t = cpool.tile([1, BSH], BF16, tag="ones")
            nc.scalar.dma_start(ones_t[:], ones_d.ap())
            b1_t = cpool.tile([1, R], BF16, tag="b1")
            b2_t = cpool.tile([1, R], BF16, tag="b2")
            b3_t = cpool.tile([1, 128], BF16, tag="b3")
            nc.scalar.dma_start(b1_t[:], b1_d.ap())
            nc.scalar.dma_start(b2_t[:], b2_d.ap())
            nc.scalar.dma_start(b3_t[:], b3_d.ap())

            # ============ attention matmuls (tensor, runs first) ============
            # two dedicated psum banks, no reuse -> no wait on vector folds
            psatt_a = psatt_pool.tile([128, 15 * A], F32, tag="psatt",
                                      name="psatt_a")
            psatt_b = psatt_pool.tile([128, 10 * A], F32, tag="psatt",
                                      name="psatt_b")

            def att_sl(mt):
                if mt < 15:
                    return psatt_a[:, mt * A:(mt + 1) * A]
                return psatt_b[:, (mt - 15) * A:(mt - 14) * A]

            for mt in range(E_MT):
                sl = att_sl(mt)
                nc.tensor.matmul(sl, ssel0[:, mt * 128:(mt + 1) * 128],
                                 bi0[:], start=True, stop=False)
                nc.tensor.matmul(sl, ssel1[:, mt * 128:(mt + 1) * 128],
                                 bi1[:], start=False, stop=True)

            # ============ stats on vector, pipelined per k-tile ============
            # bn_stats/bn_aggr per tile as the sT stream lands (keeps up
            # with arrival); the eps/sqrt/reciprocal tail is BATCHED across
            # all 7 tiles via strided views - one scalar Sqrt roundtrip
            # instead of seven (the per-tile cross-engine chain was gating
            # the entire main loop).
            mv_all = cpool.tile([128, 14], F32, tag="mvall")   # [mean,var]*7
            sg_all = cpool.tile([128, 7], F32, tag="sgall")
            rs_all = cpool.tile([128, 7], F32, tag="rsall")
            # rows 32:128 of the k=6 column are never written by bn_aggr;
            # seed them so the batched sqrt stays in range
            nc.vector.memset(mv_all[:], 1.0)
            for k in range(7):
                rows = F_T[k]
                bnst = st_pool.tile([128, 24], F32, tag="bnst",
                                    name=f"bnst_{k}")
                for g in range(4):
                    nc.vector.bn_stats(
                        bnst[0:rows, 6 * g:6 * g + 6],
                        xt[k][0:rows, 512 * g:512 * (g + 1)])
                nc.vector.bn_aggr(mv_all[0:rows, 2 * k:2 * k + 2],
                                  bnst[0:rows, :])
            var_v = mv_all[:].rearrange("p (k two) -> p k two", two=2)[:, :, 1:2]
            nc.vector.tensor_scalar(var_v, var_v, 1e-5, None,
                                    op0=mybir.AluOpType.add)
            nc.scalar.activation(sg_all[:].unsqueeze(2), var_v,
                                 mybir.ActivationFunctionType.Sqrt)
            nc.vector.reciprocal(rs_all[:], sg_all[:])
            xn = []
            for k in range(7):
                rows = F_T[k]
                xnk = xn_pool.tile([128, BSH], BF16, tag="xn", name=f"xn_{k}")
                nc.vector.tensor_scalar(
                    xnk[0:rows, :], xt[k][0:rows, 0:BSH],
                    mv_all[0:rows, 2 * k:2 * k + 1],
                    rs_all[0:rows, k:k + 1],
                    op0=mybir.AluOpType.subtract, op1=mybir.AluOpType.mult)
                xn.append(xnk)

            # ============ spn add + W1eff fold (vector) ============
            w1e_all = []
            for mt in range(E_MT):
                sl = att_sl(mt)
                spn_sl = aself_t[:, mt * A:(mt + 1) * A]
                nc.vector.tensor_tensor(out=spn_sl, in0=spn_sl, in1=sl,
                                        op=mybir.AluOpType.add)
                w1t = w1full[:, mt * R:(mt + 1) * R]
                w1e_t = w1e_pool.tile([128, R], BF16, tag="w1e",
                                      name=f"w1e_{mt}")
                nc.vector.tensor_tensor(
                    out=w1e_t[:].rearrange("p (a k) -> p a k", k=12),
                    in0=w1t.rearrange("p (a k) -> p a k", k=12),
                    in1=spn_sl.unsqueeze(2).broadcast_to((128, A, 12)),
                    op=mybir.AluOpType.mult)
                w1e_all.append(w1e_t)

            # ============ mask prep (gpsimd, after its DMA issues) ============
            act_f = cpool.tile([A, BSH], F32, tag="actf")
            nc.gpsimd.tensor_copy(act_f[:], act_i[:])
            masks = []
            for c4 in range(NA):
                mask = cpool.tile([A, BSH], F32, tag=f"mask_{c4}",
                                  name=f"mask_{c4}")
                nc.gpsimd.tensor_scalar(
                    mask[:], act_f[:], float(c4), None,
                    op0=mybir.AluOpType.is_equal)
                masks.append(mask)

            # ============ main loop: M1 + pipelined M2 ============
            h1ps = [psh1_pool.tile([128, BSH], F32, tag="h1ps",
                                   name=f"h1ps_{j}") for j in range(3)]
            embs = []

            def emit_m2(mt):
                for j, (c0, c1) in enumerate(R_SPLIT):
                    nc.tensor.matmul(h1ps[j][0:c1 - c0, :],
                                     w1e_all[mt][:, c0:c1], embs[mt][:],
                                     start=(mt == 0), stop=False)

            for mt in range(E_MT):
                ps_mt = ps_pool.tile([128, BSH], F32, tag="ps",
                                     name=f"psm_{mt}")
                for k in range(7):
                    rows = F_T[k]
                    lhsT = wfull[0:rows,
                                 k * E + mt * 128:k * E + (mt + 1) * 128]
                    nc.tensor.matmul(ps_mt[:], lhsT, xn[k][0:rows, :],
                                     start=(k == 0), stop=(k == 6))
                emb = emb_pool.tile([128, BSH], BF16, tag="emb",
                                    name=f"emb_{mt}")
                nc.scalar.activation(emb[:], ps_mt[:],
                                     mybir.ActivationFunctionType.Lrelu,
                                     alpha=0.01)
                embs.append(emb)
                if mt >= 6:
                    emit_m2(mt - 6)
            for t in range(6, 0, -1):
                emit_m2(E_MT - t)

            # finish M2: bias row, then leaky
            h1 = []
            for j, (c0, c1) in enumerate(R_SPLIT):
                w = c1 - c0
                nc.tensor.matmul(h1ps[j][0:w, :], b1_t[:, c0:c1], ones_t[:],
                                 start=False, stop=True)
                t = h_pool.tile([128, BSH], BF16, tag=f"h1_{j}",
                                name=f"h1_{j}")
                nc.scalar.activation(t[0:w, :], h1ps[j][0:w, :],
                                     mybir.ActivationFunctionType.Lrelu,
                                     alpha=0.01)
                h1.append(t)

            # M3: h2 = leaky(BD2^T @ h1 + b2)
            h2 = []
            for j, (c0, c1) in enumerate(R_SPLIT):
                w = c1 - c0
                ps3 = ps_pool.tile([128, BSH], F32, tag="ps", name=f"ps3_{j}")
                for k3, (k0, k1) in enumerate(R_SPLIT):
                    nc.tensor.matmul(ps3[0:w, :], bd2_t[k3][:, c0:c1],
                                     h1[k3][0:k1 - k0, :],
                                     start=(k3 == 0), stop=False)
                nc.tensor.matmul(ps3[0:w, :], b2_t[:, c0:c1], ones_t[:],
                                 start=False, stop=True)
                t = h_pool.tile([128, BSH], BF16, tag=f"h2_{j}",
                                name=f"h2_{j}")
                nc.scalar.activation(t[0:w, :], ps3[0:w, :],
                                     mybir.ActivationFunctionType.Lrelu,
                                     alpha=0.01)
                h2.append(t)

            # M4: all_q^T (rows = c*32+a) = BD3^T @ h2 + b3
            ps_q = ps_pool.tile([128, BSH], F32, tag="ps", name="psq")
            for k4, (k0, k1) in enumerate(R_SPLIT):
                nc.tensor.matmul(ps_q[:], bd3_t[k4][:, :],
                                 h2[k4][0:k1 - k0, :],
                                 start=(k4 == 0), stop=False)
            nc.tensor.matmul(ps_q[:], b3_t[:], ones_t[:], start=False,
                             stop=True)

            # gather: q[a, b] = all_q[c(a,b)*32+a, b] via one-hot masks
            qs = []
            for c4 in range(NA):
                qc = cpool.tile([A, BSH], F32, tag=f"qc_{c4}",
                                name=f"qc_{c4}")
                nc.vector.tensor_tensor(
                    out=qc[:], in0=ps_q[c4 * 32:c4 * 32 + A, :],
                    in1=masks[c4][:], op=mybir.AluOpType.mult)
                qs.append(qc)
            nc.vector.tensor_tensor(out=qs[0][:], in0=qs[0][:], in1=qs[1][:],
                                    op=mybir.AluOpType.add)
            nc.vector.tensor_tensor(out=qs[2][:], in0=qs[2][:], in1=qs[3][:],
                                    op=mybir.AluOpType.add)
            nc.vector.tensor_tensor(out=qs[0][:], in0=qs[0][:], in1=qs[2][:],
                                    op=mybir.AluOpType.add)
            nc.sync.dma_start(out_d.ap(), qs[0][:])

    nc.compile()
    return nc


def _host_prep(inputs):
    states = np.asarray(inputs["states"], dtype=np.float32)
    ehh_w = np.asarray(inputs["ehh_w"], dtype=np.float32)
    anova = np.asarray(inputs["anova"], dtype=np.float32)
    w1 = np.asarray(inputs["w1"], dtype=np.float32)
    b1 = np.asarray(inputs["b1"], dtype=np.float32)
    w2 = np.asarray(inputs["w2"], dtype=np.float32)
    b2 = np.asarray(inputs["b2"], dtype=np.float32)
    w3 = np.asarray(inputs["w3"], dtype=np.float32)
    b3 = np.asarray(inputs["b3"], dtype=np.float32)
    actions = np.asarray(inputs["actions"], dtype=np.int32)
    adj = np.asarray(inputs["adj"], dtype=np.int64)

    sT = np.ascontiguousarray(
        states.transpose(0, 2, 1).reshape(F, B)).astype(BF)
    w1T = w1.transpose(1, 0, 2).reshape(E, R)
    # partition-major rearranges (exact SBUF layout -> simple 2D DMAs)
    w1R = np.ascontiguousarray(
        w1T.reshape(E_MT, 128, R).transpose(1, 0, 2).reshape(128, E_MT * R)
    ).astype(BF)
    aselfR = np.ascontiguousarray(
        anova[:E].reshape(E_MT, 128, A).transpose(1, 0, 2)
        .reshape(128, E_MT * A)).astype(BF)

    # adjacency scatter -> winning source row per target (last write wins,
    # col-3 scatter applied after col-1 scatter)
    src = np.full(E, -1, dtype=np.int64)
    for e in range(adj.shape[0]):
        src[adj[e, 1]] = adj[e, 0]
    for e in range(adj.shape[0]):
        src[adj[e, 3]] = adj[e, 0]
    ssel = np.zeros((INTER, E), dtype=np.uint8)
    hit = np.nonzero(src >= 0)[0]
    ssel[src[hit], hit] = 1

    bd2 = np.zeros((R, R), dtype=np.float32)
    bd3 = np.zeros((R, 128), dtype=np.float32)
    b3r = np.zeros((1, 128), dtype=np.float32)
    for a in range(A):
        bd2[12 * a:12 * a + 12, 12 * a:12 * a + 12] = w2[a]
        for c in range(NA):
            bd3[12 * a:12 * a + 12, c * 32 + a] = w3[a, :, c]
            b3r[0, c * 32 + a] = b3[a, c]

    common = {
        "ehh_w": np.ascontiguousarray(ehh_w).astype(BF),
        "w1R": w1R,
        "ssel": ssel,
        "aselfR": aselfR,
        "bi": np.ascontiguousarray(anova[E:]).astype(BF),
        "bd2": bd2.astype(BF),
        "bd3": bd3.astype(BF),
        "b1r": b1.reshape(1, R).astype(BF),
        "b2r": b2.reshape(1, R).astype(BF),
        "b3r": b3r.astype(BF),
        "ones": np.ones((1, BSH), dtype=BF),
    }
    in_maps = []
    for c in range(N_CORES):
        m = dict(common)
        m["sT"] = np.ascontiguousarray(np.roll(sT, -BSH * c, axis=1))
        m["act"] = np.ascontiguousarray(actions[:, BSH * c:BSH * (c + 1)])
        in_maps.append(m)
    return in_maps


def kernel(**inputs):
    global LAST_EXEC_NS
    if "nc" not in _CACHE:
        _CACHE["nc"] = _build_program()
    nc = _CACHE["nc"]
    in_maps = _host_prep(inputs)
    kwargs = {}
    if TRACE:
        kwargs["trace"] = True
    res = bass_utils.run_bass_kernel_spmd(
        nc, in_maps, core_ids=list(range(N_CORES)), **kwargs)
    LAST_EXEC_NS = res.exec_time_ns
    q = np.empty((A, B), dtype=np.float32)
    for c in range(N_CORES):
        q[:, BSH * c:BSH * (c + 1)] = res.results[c]["out"]
    return q


# revision 26
# speedup vs baseline: 1.2746x; 1.0337x over previous
"""Trainium2 Bass kernel for nn_BRGEHHNet (gnn_message_passing).

Contract: kernel(**inputs) takes FULL unsharded inputs (as produced by
setup_inputs) and returns the FULL (25, 2048) float32 output.

Strategy: data-parallel over the batch dim across 8 NeuronCores.
Each core handles a 256-column batch shard; the small anova/adjacency
and per-agent critic weights are replicated. BatchNorm statistics are
over the full batch, so every core loads the full transposed states
and computes the stats locally (no collectives - collective latency
floor of ~7us would dominate this kernel).

v2 over the 95.6us baseline:
  - all weights + states shipped host-side as bf16 (halves HBM traffic
    to ~11 MB/core; rel-err budget 2e-2 tolerates it)
  - DMA issue spread across sync/scalar/gpsimd queues so no queue
    serializes the stream
  - stats pipelined per 128-feature tile as sT tiles land
  - attention psum lives in two dedicated banks (no bank reuse ->
    att matmuls never wait on the vector folds)
  - mask prep moved to gpsimd; vector does stats + folds only

Math notes:
  - att_out = emb * all_att broadcast followed by per-agent critics is
    folded into the critic weights: h1 = emb @ (all_att.T expanded * w1T).
  - The adjacency scatter (last-write-wins, matching jax-CPU/torch
    semantics) is encoded host-side as a 0/1 selection matrix S_sel
    (150 x 3200, uint8); neighbor_att = S_sel[:, r].T @ bi_att on the PE.
  - w2/w3 per-agent critics become block-diagonal matmuls; biases are
    folded into the PSUM accumulation as K=1 matmuls against a ones row.
  - The action gather is a one-hot mask multiply on the vector engine.
"""

import os
import numpy as np
import ml_dtypes

import concourse.bacc as bacc
import concourse.mybir as mybir
import concourse.tile as tile
from concourse import bass_utils

N_CORES = 8
A = 25          # agents
B = 2048        # batch
S = 32          # state dim
F = A * S       # 800 features (contraction of M1)
E = 3200        # EHH_HID (= 25 * 128)
R = A * 12      # 300 critic hidden rows
INTER = 150
NA = 4
BSH = B // N_CORES  # 256 per-core batch shard

F_T = [128] * 6 + [32]          # feature tiles (800 = 6*128 + 32)
E_MT = E // 128                  # 25 output tiles of M1
R_SPLIT = [(0, 128), (128, 256), (256, 300)]   # (a,k) row tiling
WCH = 1024                       # ehh_w column-chunk width (8 mt per chunk)

DT = mybir.dt
F32 = DT.float32
BF16 = DT.bfloat16
I32 = DT.int32
U8 = DT.uint8

TRACE = os.environ.get("BASS_KERNEL_TRACE", "0") == "1"
LAST_EXEC_NS = None

_CACHE = {}

BF = ml_dtypes.bfloat16


def _build_program():
    nc = bacc.Bacc("TRN2", target_bir_lowering=False, debug=False,
                   num_devices=N_CORES)

    sT_d = nc.dram_tensor("sT", [F, B], BF16, kind="ExternalInput")
    ehh_w_d = nc.dram_tensor("ehh_w", [F, E], BF16, kind="ExternalInput")
    # w1R / aselfR are host-rearranged into the exact SBUF layout
    # (partition-major), so their DMAs are single simple 2D patterns.
    w1R_d = nc.dram_tensor("w1R", [128, E_MT * R], BF16, kind="ExternalInput")
    ssel_d = nc.dram_tensor("ssel", [INTER, E], U8, kind="ExternalInput")
    aselfR_d = nc.dram_tensor("aselfR", [128, E_MT * A], BF16,
                              kind="ExternalInput")
    bi_d = nc.dram_tensor("bi", [INTER, A], BF16, kind="ExternalInput")
    bd2_d = nc.dram_tensor("bd2", [R, R], BF16, kind="ExternalInput")
    bd3_d = nc.dram_tensor("bd3", [R, 128], BF16, kind="ExternalInput")
    b1_d = nc.dram_tensor("b1r", [1, R], BF16, kind="ExternalInput")
    b2_d = nc.dram_tensor("b2r", [1, R], BF16, kind="ExternalInput")
    b3_d = nc.dram_tensor("b3r", [1, 128], BF16, kind="ExternalInput")
    ones_d = nc.dram_tensor("ones", [1, BSH], BF16, kind="ExternalInput")
    act_d = nc.dram_tensor("act", [A, BSH], I32, kind="ExternalInput")
    out_d = nc.dram_tensor("out", [A, BSH], F32, kind="ExternalOutput")

    with tile.TileContext(nc) as tc:
        with (
            tc.tile_pool(name="const", bufs=1) as cpool,
            tc.tile_pool(name="xt", bufs=7) as xt_pool,
            tc.tile_pool(name="xn", bufs=7) as xn_pool,
            tc.tile_pool(name="st", bufs=7) as st_pool,
            tc.tile_pool(name="big", bufs=1) as big_pool,
            tc.tile_pool(name="w1e", bufs=25) as w1e_pool,
            tc.tile_pool(name="emb", bufs=12) as emb_pool,
            tc.tile_pool(name="hh", bufs=8) as h_pool,
            tc.tile_pool(name="ps", bufs=3, space="PSUM") as ps_pool,
            tc.tile_pool(name="psatt", bufs=2, space="PSUM") as psatt_pool,
            tc.tile_pool(name="psh1", bufs=3, space="PSUM") as psh1_pool,
        ):
            # ================= DMA issue =================
            # sync queue: the sT stream first (stats critical path; queue
            # order gives it priority over the big weight chunks behind it),
            # then the ehh_w column-chunks, then late-need small tensors.
            # stripe the sT tiles across TWO queues: a single DMA queue
            # tops out near ~140 GB/s, far below the 358 GB/s HBM rate,
            # and the sT arrival paces the entire prefix
            xt = []
            for k in range(7):
                rows = F_T[k]
                t = xt_pool.tile([128, B], BF16, tag="xt", name=f"xt_{k}")
                eng = nc.sync if k % 2 == 0 else nc.scalar
                eng.dma_start(t[0:rows, :],
                              sT_d.ap()[k * 128:k * 128 + rows, :])
                xt.append(t)
            act_i = cpool.tile([A, BSH], I32, tag="acti")
            nc.sync.dma_start(act_i[:], act_d.ap())
            bd2_t = []
            for j, (c0, c1) in enumerate(R_SPLIT):
                t = cpool.tile([c1 - c0, R], BF16, tag=f"bd2_{j}",
                               name=f"bd2t_{j}")
                nc.sync.dma_start(t[:], bd2_d.ap()[c0:c1, :])
                bd2_t.append(t)
            bd3_t = []
            for j, (c0, c1) in enumerate(R_SPLIT):
                t = cpool.tile([c1 - c0, 128], BF16, tag=f"bd3_{j}",
                               name=f"bd3t_{j}")
                nc.sync.dma_start(t[:], bd3_d.ap()[c0:c1, :])
                bd3_t.append(t)

            # gpsimd queue: ssel (casting DMA: gpsimd-only), then the small
            # attention tables and the w1R slices (all simple 2D patterns).
            ssel0 = cpool.tile([128, E], BF16, tag="ssel0")
            ssel1 = cpool.tile([INTER - 128, E], BF16, tag="ssel1")
            nc.gpsimd.dma_start(ssel0[:], ssel_d.ap()[0:128, :])
            nc.gpsimd.dma_start(ssel1[:], ssel_d.ap()[128:INTER, :])
            aself_t = cpool.tile([128, E_MT * A], BF16, tag="aselfR")
            nc.gpsimd.dma_start(aself_t[:], aselfR_d.ap())
            bi0 = cpool.tile([128, A], BF16, tag="bi0")
            bi1 = cpool.tile([INTER - 128, A], BF16, tag="bi1")
            nc.gpsimd.dma_start(bi0[:], bi_d.ap()[0:128, :])
            nc.gpsimd.dma_start(bi1[:], bi_d.ap()[128:INTER, :])
            # PRIORITY GATE: the sT stream owns the HBM until the stats
            # inputs have landed — hold the big weight streams back by making
            # the gpsimd queue wait on the second-to-last sT tile.
            gate_scr = cpool.tile([1, 64], BF16, tag="gate")
            nc.gpsimd.tensor_copy(gate_scr[0:1, :], xt[5][0:1, 0:64])
            # gpsimd also carries the big weight streams, column-chunked so
            # mm1/mm2 can start after the first chunk; w1R slices interleave
            wfull = big_pool.tile([128, 7 * E], BF16, tag="wfull")
            w1full = big_pool.tile([128, E_MT * R], BF16, tag="w1full")
            NCH = 5
            CW = E // NCH      # 640-column chunks (5 mt each)
            for g in range(NCH):
                g0 = g * CW
                g1 = g0 + CW
                nc.gpsimd.dma_start(
                    wfull[:].rearrange("p (k c) -> p k c", c=E)[:, 0:6, g0:g1],
                    ehh_w_d.ap()[0:768, g0:g1]
                    .rearrange("(k p) c -> p k c", p=128))
                nc.gpsimd.dma_start(
                    wfull[0:32, 6 * E + g0:6 * E + g1],
                    ehh_w_d.ap()[768:800, g0:g1])
                nc.gpsimd.dma_start(
                    w1full[:, g * 5 * R:(g + 1) * 5 * R],
                    w1R_d.ap()[:, g * 5 * R:(g + 1) * 5 * R])

            # scalar queue: tiny rows (simple, fast issues), then sqrt +
            # leaky work arrives behind them.
            ones_t = cpool.tile([1, BSH], BF16, tag="ones")
            nc.scalar.dma_start(ones_t[:], ones_d.ap())
            b1_t = cpool.tile([1, R], BF16, tag="b1")
            b2_t = cpool.tile([1, R], BF16, tag="b2")
            b3_t = cpool.tile([1, 128], BF16, tag="b3")
            nc.scalar.dma_start(b1_t[:], b1_d.ap())
            nc.scalar.dma_start(b2_t[:], b2_d.ap())
            nc.scalar.dma_start(b3_t[:], b3_d.ap())

            # ============ attention matmuls (tensor, runs first) ============
            # two dedicated psum banks, no reuse -> no wait on vector folds
            psatt_a = psatt_pool.tile([128, 15 * A], F32, tag="psatt",
                                      name="psatt_a")
            psatt_b = psatt_pool.tile([128, 10 * A], F32, tag="psatt",
                                      name="psatt_b")

            def att_sl(mt):
                if mt < 15:
                    return psatt_a[:, mt * A:(mt + 1) * A]
                return psatt_b[:, (mt - 15) * A:(mt - 14) * A]

            for mt in range(E_MT):
                sl = att_sl(mt)
                nc.tensor.matmul(sl, ssel0[:, mt * 128:(mt + 1) * 128],
                                 bi0[:], start=True, stop=False)
                nc.tensor.matmul(sl, ssel1[:, mt * 128:(mt + 1) * 128],
                                 bi1[:], start=False, stop=True)

            # ============ stats on vector, pipelined per k-tile ============
            xn = []
            for k in range(7):
                rows = F_T[k]
                ssum = st_pool.tile([128, 4], F32, tag="st", name=f"ssum_{k}")
                bnst = st_pool.tile([128, 24], F32, tag="bnst",
                                    name=f"bnst_{k}")
                for g in range(4):
                    nc.vector.bn_stats(
                        bnst[0:rows, 6 * g:6 * g + 6],
                        xt[k][0:rows, 512 * g:512 * (g + 1)])
                nc.vector.bn_aggr(ssum[0:rows, 0:2], bnst[0:rows, :])
                nc.vector.tensor_scalar(
                    ssum[0:rows, 1:2], ssum[0:rows, 1:2], 1e-5, None,
                    op0=mybir.AluOpType.add)
                nc.scalar.activation(
                    ssum[0:rows, 2:3], ssum[0:rows, 1:2],
                    mybir.ActivationFunctionType.Sqrt)
                nc.vector.reciprocal(ssum[0:rows, 3:4], ssum[0:rows, 2:3])
                xnk = xn_pool.tile([128, BSH], BF16, tag="xn", name=f"xn_{k}")
                nc.vector.tensor_scalar(
                    xnk[0:rows, :], xt[k][0:rows, 0:BSH],
                    ssum[0:rows, 0:1], ssum[0:rows, 3:4],
                    op0=mybir.AluOpType.subtract, op1=mybir.AluOpType.mult)
                xn.append(xnk)

            # ============ spn add + W1eff fold (vector) ============
            w1e_all = []
            for mt in range(E_MT):
                sl = att_sl(mt)
                spn_sl = aself_t[:, mt * A:(mt + 1) * A]
                nc.vector.tensor_tensor(out=spn_sl, in0=spn_sl, in1=sl,
                                        op=mybir.AluOpType.add)
                w1t = w1full[:, mt * R:(mt + 1) * R]
                w1e_t = w1e_pool.tile([128, R], BF16, tag="w1e",
                                      name=f"w1e_{mt}")
                nc.vector.tensor_tensor(
                    out=w1e_t[:].rearrange("p (a k) -> p a k", k=12),
                    in0=w1t.rearrange("p (a k) -> p a k", k=12),
                    in1=spn_sl.unsqueeze(2).broadcast_to((128, A, 12)),
                    op=mybir.AluOpType.mult)
                w1e_all.append(w1e_t)

            # ============ mask prep (gpsimd, after its DMA issues) ============
            act_f = cpool.tile([A, BSH], F32, tag="actf")
            nc.gpsimd.tensor_copy(act_f[:], act_i[:])
            masks = []
            for c4 in range(NA):
                mask = cpool.tile([A, BSH], F32, tag=f"mask_{c4}",
                                  name=f"mask_{c4}")
                nc.gpsimd.tensor_scalar(
                    mask[:], act_f[:], float(c4), None,
                    op0=mybir.AluOpType.is_equal)
                masks.append(mask)

            # ============ main loop: M1 + pipelined M2 ============
            h1ps = [psh1_pool.tile([128, BSH], F32, tag="h1ps",
                                   name=f"h1ps_{j}") for j in range(3)]
            embs = []

            def emit_m2(mt):
                for j, (c0, c1) in enumerate(R_SPLIT):
                    nc.tensor.matmul(h1ps[j][0:c1 - c0, :],
                                     w1e_all[mt][:, c0:c1], embs[mt][:],
                                     start=(mt == 0), stop=False)

            for mt in range(E_MT):
                ps_mt = ps_pool.tile([128, BSH], F32, tag="ps",
                                     name=f"psm_{mt}")
                for k in range(7):
                    rows = F_T[k]
                    lhsT = wfull[0:rows,
                                 k * E + mt * 128:k * E + (mt + 1) * 128]
                    nc.tensor.matmul(ps_mt[:], lhsT, xn[k][0:rows, :],
                                     start=(k == 0), stop=(k == 6))
                emb = emb_pool.tile([128, BSH], BF16, tag="emb",
                                    name=f"emb_{mt}")
                nc.scalar.activation(emb[:], ps_mt[:],
                                     mybir.ActivationFunctionType.Lrelu,
                                     alpha=0.01)
                embs.append(emb)
                if mt >= 6:
                    emit_m2(mt - 6)
            for t in range(6, 0, -1):
                emit_m2(E_MT - t)

            # finish M2: bias row, then leaky
            h1 = []
            for j, (c0, c1) in enumerate(R_SPLIT):
                w = c1 - c0
                nc.tensor.matmul(h1ps[j][0:w, :], b1_t[:, c0:c1], ones_t[:],
                                 start=False, stop=True)
                t = h_pool.tile([128, BSH], BF16, tag=f"h1_{j}",
                                name=f"h1_{j}")
                nc.scalar.activation(t[0:w, :], h1ps[j][0:w, :],
                                     mybir.ActivationFunctionType.Lrelu,
                                     alpha=0.01)
                h1.append(t)

            # M3: h2 = leaky(BD2^T @ h1 + b2)
            h2 = []
            for j, (c0, c1) in enumerate(R_SPLIT):
                w = c1 - c0
                ps3 = ps_pool.tile([128, BSH], F32, tag="ps", name=f"ps3_{j}")
                for k3, (k0, k1) in enumerate(R_SPLIT):
                    nc.tensor.matmul(ps3[0:w, :], bd2_t[k3][:, c0:c1],
                                     h1[k3][0:k1 - k0, :],
                                     start=(k3 == 0), stop=False)
                nc.tensor.matmul(ps3[0:w, :], b2_t[:, c0:c1], ones_t[:],
                                 start=False, stop=True)
                t = h_pool.tile([128, BSH], BF16, tag=f"h2_{j}",
                                name=f"h2_{j}")
                nc.scalar.activation(t[0:w, :], ps3[0:w, :],
                                     mybir.ActivationFunctionType.Lrelu,
                                     alpha=0.01)
                h2.append(t)

            # M4: all_q^T (rows = c*32+a) = BD3^T @ h2 + b3
            ps_q = ps_pool.tile([128, BSH], F32, tag="ps", name="psq")
            for k4, (k0, k1) in enumerate(R_SPLIT):
                nc.tensor.matmul(ps_q[:], bd3_t[k4][:, :],
                                 h2[k4][0:k1 - k0, :],
                                 start=(k4 == 0), stop=False)
            nc.tensor.matmul(ps_q[:], b3_t[:], ones_t[:], start=False,
                             stop=True)

            # gather: q[a, b] = all_q[c(a,b)*32+a, b] via one-hot masks
            qs = []
            for c4 in range(NA):
                qc = cpool.tile([A, BSH], F32, tag=f"qc_{c4}",
                                name=f"qc_{c4}")
                nc.vector.tensor_tensor(
                    out=qc[:], in0=ps_q[c4 * 32:c4 * 32 + A, :],
                    in1=masks[c4][:], op=mybir.AluOpType.mult)
                qs.append(qc)
            nc.vector.tensor_tensor(out=qs[0][:], in0=qs[0][:], in1=qs[1][:],
                                    op=mybir.AluOpType.add)
            nc.vector.tensor_tensor(out=qs[2][:], in0=qs[2][:], in1=qs[3][:],
                                    op=mybir.AluOpType.add)
            nc.vector.tensor_tensor(out=qs[0][:], in0=qs[0][:], in1=qs[2][:],
                                    op=mybir.AluOpType.add)
            nc.sync.dma_start(out_d.ap(), qs[0][:])

    nc.compile()
    return nc


def _host_prep(inputs):
    states = np.asarray(inputs["states"], dtype=np.float32)
    ehh_w = np.asarray(inputs["ehh_w"], dtype=np.float32)
    anova = np.asarray(inputs["anova"], dtype=np.float32)
    w1 = np.asarray(inputs["w1"], dtype=np.float32)
    b1 = np.asarray(inputs["b1"], dtype=np.float32)
    w2 = np.asarray(inputs["w2"], dtype=np.float32)
    b2 = np.asarray(inputs["b2"], dtype=np.float32)
    w3 = np.asarray(inputs["w3"], dtype=np.float32)
    b3 = np.asarray(inputs["b3"], dtype=np.float32)
    actions = np.asarray(inputs["actions"], dtype=np.int32)
    adj = np.asarray(inputs["adj"], dtype=np.int64)

    sT = np.ascontiguousarray(
        states.transpose(0, 2, 1).reshape(F, B)).astype(BF)
    w1T = w1.transpose(1, 0, 2).reshape(E, R)
    # partition-major rearranges (exact SBUF layout -> simple 2D DMAs)
    w1R = np.ascontiguousarray(
        w1T.reshape(E_MT, 128, R).transpose(1, 0, 2).reshape(128, E_MT * R)
    ).astype(BF)
    aselfR = np.ascontiguousarray(
        anova[:E].reshape(E_MT, 128, A).transpose(1, 0, 2)
        .reshape(128, E_MT * A)).astype(BF)

    # adjacency scatter -> winning source row per target (last write wins,
    # col-3 scatter applied after col-1 scatter)
    src = np.full(E, -1, dtype=np.int64)
    for e in range(adj.shape[0]):
        src[adj[e, 1]] = adj[e, 0]
    for e in range(adj.shape[0]):
        src[adj[e, 3]] = adj[e, 0]
    ssel = np.zeros((INTER, E), dtype=np.uint8)
    hit = np.nonzero(src >= 0)[0]
    ssel[src[hit], hit] = 1

    bd2 = np.zeros((R, R), dtype=np.float32)
    bd3 = np.zeros((R, 128), dtype=np.float32)
    b3r = np.zeros((1, 128), dtype=np.float32)
    for a in range(A):
        bd2[12 * a:12 * a + 12, 12 * a:12 * a + 12] = w2[a]
        for c in range(NA):
            bd3[12 * a:12 * a + 12, c * 32 + a] = w3[a, :, c]
            b3r[0, c * 32 + a] = b3[a, c]

    common = {
        "ehh_w": np.ascontiguousarray(ehh_w).astype(BF),
        "w1R": w1R,
        "ssel": ssel,
        "aselfR": aselfR,
        "bi": np.ascontiguousarray(anova[E:]).astype(BF),
        "bd2": bd2.astype(BF),
        "bd3": bd3.astype(BF),
        "b1r": b1.reshape(1, R).astype(BF),
        "b2r": b2.reshape(1, R).astype(BF),
        "b3r": b3r.astype(BF),
        "ones": np.ones((1, BSH), dtype=BF),
    }
    in_maps = []
    for c in range(N_CORES):
        m = dict(common)
        m["sT"] = np.ascontiguousarray(np.roll(sT, -BSH * c, axis=1))
        m["act"] = np.ascontiguousarray(actions[:, BSH * c:BSH * (c + 1)])
        in_maps.append(m)
    return in_maps


def kernel(**inputs):
    global LAST_EXEC_NS
    if "nc" not in _CACHE:
        _CACHE["nc"] = _build_program()
    nc = _CACHE["nc"]
    in_maps = _host_prep(inputs)
    kwargs = {}
    if TRACE:
        kwargs["trace"] = True
    res = bass_utils.run_bass_kernel_spmd(
        nc, in_maps, core_ids=list(range(N_CORES)), **kwargs)
    LAST_EXEC_NS = res.exec_time_ns
    q = np.empty((A, B), dtype=np.float32)
    for c in range(N_CORES):
        q[:, BSH * c:BSH * (c + 1)] = res.results[c]["out"]
    return q


# revision 29
# speedup vs baseline: 1.3067x; 1.0252x over previous
"""Trainium2 Bass kernel for nn_BRGEHHNet (gnn_message_passing).

Contract: kernel(**inputs) takes FULL unsharded inputs (as produced by
setup_inputs) and returns the FULL (25, 2048) float32 output.

Strategy: data-parallel over the batch dim across 8 NeuronCores.
Each core handles a 256-column batch shard; the small anova/adjacency
and per-agent critic weights are replicated. BatchNorm statistics are
over the full batch, so every core loads the full transposed states
and computes the stats locally (no collectives - collective latency
floor of ~7us would dominate this kernel).

v2 over the 95.6us baseline:
  - all weights + states shipped host-side as bf16 (halves HBM traffic
    to ~11 MB/core; rel-err budget 2e-2 tolerates it)
  - DMA issue spread across sync/scalar/gpsimd queues so no queue
    serializes the stream
  - stats pipelined per 128-feature tile as sT tiles land
  - attention psum lives in two dedicated banks (no bank reuse ->
    att matmuls never wait on the vector folds)
  - mask prep moved to gpsimd; vector does stats + folds only

Math notes:
  - att_out = emb * all_att broadcast followed by per-agent critics is
    folded into the critic weights: h1 = emb @ (all_att.T expanded * w1T).
  - The adjacency scatter (last-write-wins, matching jax-CPU/torch
    semantics) is encoded host-side as a 0/1 selection matrix S_sel
    (150 x 3200, uint8); neighbor_att = S_sel[:, r].T @ bi_att on the PE.
  - w2/w3 per-agent critics become block-diagonal matmuls; biases are
    folded into the PSUM accumulation as K=1 matmuls against a ones row.
  - The action gather is a one-hot mask multiply on the vector engine.
"""

import os
import numpy as np
import ml_dtypes

import concourse.bacc as bacc
import concourse.mybir as mybir
import concourse.tile as tile
from concourse import bass_utils

N_CORES = 8
A = 25          # agents
B = 2048        # batch
S = 32          # state dim
F = A * S       # 800 features (contraction of M1)
E = 3200        # EHH_HID (= 25 * 128)
R = A * 12      # 300 critic hidden rows
INTER = 150
NA = 4
BSH = B // N_CORES  # 256 per-core batch shard

F_T = [128] * 6 + [32]          # feature tiles (800 = 6*128 + 32)
E_MT = E // 128                  # 25 output tiles of M1
R_SPLIT = [(0, 128), (128, 256), (256, 300)]   # (a,k) row tiling
WCH = 1024                       # ehh_w column-chunk width (8 mt per chunk)

DT = mybir.dt
F32 = DT.float32
BF16 = DT.bfloat16
I32 = DT.int32
U8 = DT.uint8

TRACE = os.environ.get("BASS_KERNEL_TRACE", "0") == "1"
LAST_EXEC_NS = None

_CACHE = {}

BF = ml_dtypes.bfloat16


def _build_program():
    nc = bacc.Bacc("TRN2", target_bir_lowering=False, debug=False,
                   num_devices=N_CORES)

    sT_d = nc.dram_tensor("sT", [F, B], BF16, kind="ExternalInput")
    ehh_w_d = nc.dram_tensor("ehh_w", [F, E], BF16, kind="ExternalInput")
    # w1R / aselfR are host-rearranged into the exact SBUF layout
    # (partition-major), so their DMAs are single simple 2D patterns.
    w1R_d = nc.dram_tensor("w1R", [128, E_MT * R], BF16, kind="ExternalInput")
    ssel_d = nc.dram_tensor("ssel", [INTER, E], U8, kind="ExternalInput")
    aselfR_d = nc.dram_tensor("aselfR", [128, E_MT * A], BF16,
                              kind="ExternalInput")
    bi_d = nc.dram_tensor("bi", [INTER, A], BF16, kind="ExternalInput")
    bd2_d = nc.dram_tensor("bd2", [R, R], BF16, kind="ExternalInput")
    bd3_d = nc.dram_tensor("bd3", [R, 128], BF16, kind="ExternalInput")
    b1_d = nc.dram_tensor("b1r", [1, R], BF16, kind="ExternalInput")
    b2_d = nc.dram_tensor("b2r", [1, R], BF16, kind="ExternalInput")
    b3_d = nc.dram_tensor("b3r", [1, 128], BF16, kind="ExternalInput")
    ones_d = nc.dram_tensor("ones", [1, BSH], BF16, kind="ExternalInput")
    act_d = nc.dram_tensor("act", [A, BSH], I32, kind="ExternalInput")
    out_d = nc.dram_tensor("out", [A, BSH], F32, kind="ExternalOutput")

    with tile.TileContext(nc) as tc:
        with (
            tc.tile_pool(name="const", bufs=1) as cpool,
            tc.tile_pool(name="xt", bufs=7) as xt_pool,
            tc.tile_pool(name="xn", bufs=7) as xn_pool,
            tc.tile_pool(name="st", bufs=7) as st_pool,
            tc.tile_pool(name="big", bufs=1) as big_pool,
            tc.tile_pool(name="w1e", bufs=25) as w1e_pool,
            tc.tile_pool(name="emb", bufs=25) as emb_pool,
            tc.tile_pool(name="hh", bufs=8) as h_pool,
            tc.tile_pool(name="ps", bufs=3, space="PSUM") as ps_pool,
            tc.tile_pool(name="psatt", bufs=2, space="PSUM") as psatt_pool,
            tc.tile_pool(name="psh1", bufs=3, space="PSUM") as psh1_pool,
        ):
            # ================= DMA issue =================
            # sync queue: the sT stream first (stats critical path; queue
            # order gives it priority over the big weight chunks behind it),
            # then the ehh_w column-chunks, then late-need small tensors.
            xt = []
            for k in range(7):
                rows = F_T[k]
                t = xt_pool.tile([128, B], BF16, tag="xt", name=f"xt_{k}")
                nc.sync.dma_start(t[0:rows, :],
                                  sT_d.ap()[k * 128:k * 128 + rows, :])
                xt.append(t)
            act_i = cpool.tile([A, BSH], I32, tag="acti")
            nc.sync.dma_start(act_i[:], act_d.ap())
            bd2_t = []
            for j, (c0, c1) in enumerate(R_SPLIT):
                t = cpool.tile([c1 - c0, R], BF16, tag=f"bd2_{j}",
                               name=f"bd2t_{j}")
                nc.sync.dma_start(t[:], bd2_d.ap()[c0:c1, :])
                bd2_t.append(t)
            bd3_t = []
            for j, (c0, c1) in enumerate(R_SPLIT):
                t = cpool.tile([c1 - c0, 128], BF16, tag=f"bd3_{j}",
                               name=f"bd3t_{j}")
                nc.sync.dma_start(t[:], bd3_d.ap()[c0:c1, :])
                bd3_t.append(t)

            # gpsimd queue: ssel (casting DMA: gpsimd-only), then the small
            # attention tables and the w1R slices (all simple 2D patterns).
            ssel0 = cpool.tile([128, E], BF16, tag="ssel0")
            ssel1 = cpool.tile([INTER - 128, E], BF16, tag="ssel1")
            nc.gpsimd.dma_start(ssel0[:], ssel_d.ap()[0:128, :])
            nc.gpsimd.dma_start(ssel1[:], ssel_d.ap()[128:INTER, :])
            aself_t = cpool.tile([128, E_MT * A], BF16, tag="aselfR")
            nc.gpsimd.dma_start(aself_t[:], aselfR_d.ap())
            bi0 = cpool.tile([128, A], BF16, tag="bi0")
            bi1 = cpool.tile([INTER - 128, A], BF16, tag="bi1")
            nc.gpsimd.dma_start(bi0[:], bi_d.ap()[0:128, :])
            nc.gpsimd.dma_start(bi1[:], bi_d.ap()[128:INTER, :])
            # PRIORITY GATE: the sT stream owns the HBM until the stats
            # inputs have landed — hold the big weight streams back by making
            # the gpsimd queue wait on the second-to-last sT tile.
            gate_scr = cpool.tile([1, 64], BF16, tag="gate")
            nc.gpsimd.tensor_copy(gate_scr[0:1, :], xt[5][0:1, 0:64])
            # gpsimd also carries the big weight streams, column-chunked so
            # mm1/mm2 can start after the first chunk; w1R slices interleave
            wfull = big_pool.tile([128, 7 * E], BF16, tag="wfull")
            w1full = big_pool.tile([128, E_MT * R], BF16, tag="w1full")
            NCH = 5
            CW = E // NCH      # 640-column chunks (5 mt each)
            for g in range(NCH):
                g0 = g * CW
                g1 = g0 + CW
                nc.gpsimd.dma_start(
                    wfull[:].rearrange("p (k c) -> p k c", c=E)[:, 0:6, g0:g1],
                    ehh_w_d.ap()[0:768, g0:g1]
                    .rearrange("(k p) c -> p k c", p=128))
                nc.gpsimd.dma_start(
                    wfull[0:32, 6 * E + g0:6 * E + g1],
                    ehh_w_d.ap()[768:800, g0:g1])
                nc.gpsimd.dma_start(
                    w1full[:, g * 5 * R:(g + 1) * 5 * R],
                    w1R_d.ap()[:, g * 5 * R:(g + 1) * 5 * R])

            # scalar queue: tiny rows (simple, fast issues), then sqrt +
            # leaky work arrives behind them.
            ones_t = cpool.tile([1, BSH], BF16, tag="ones")
            nc.scalar.dma_start(ones_t[:], ones_d.ap())
            b1_t = cpool.tile([1, R], BF16, tag="b1")
            b2_t = cpool.tile([1, R], BF16, tag="b2")
            b3_t = cpool.tile([1, 128], BF16, tag="b3")
            nc.scalar.dma_start(b1_t[:], b1_d.ap())
            nc.scalar.dma_start(b2_t[:], b2_d.ap())
            nc.scalar.dma_start(b3_t[:], b3_d.ap())

            # ============ attention matmuls (tensor, runs first) ============
            # two dedicated psum banks, no reuse -> no wait on vector folds
            psatt_a = psatt_pool.tile([128, 15 * A], F32, tag="psatt",
                                      name="psatt_a")
            psatt_b = psatt_pool.tile([128, 10 * A], F32, tag="psatt",
                                      name="psatt_b")

            def att_sl(mt):
                if mt < 15:
                    return psatt_a[:, mt * A:(mt + 1) * A]
                return psatt_b[:, (mt - 15) * A:(mt - 14) * A]

            for mt in range(E_MT):
                sl = att_sl(mt)
                nc.tensor.matmul(sl, ssel0[:, mt * 128:(mt + 1) * 128],
                                 bi0[:], start=True, stop=False)
                nc.tensor.matmul(sl, ssel1[:, mt * 128:(mt + 1) * 128],
                                 bi1[:], start=False, stop=True)

            # ============ stats on vector, pipelined per k-tile ============
            xn = []
            for k in range(7):
                rows = F_T[k]
                ssum = st_pool.tile([128, 4], F32, tag="st", name=f"ssum_{k}")
                bnst = st_pool.tile([128, 24], F32, tag="bnst",
                                    name=f"bnst_{k}")
                for g in range(4):
                    nc.vector.bn_stats(
                        bnst[0:rows, 6 * g:6 * g + 6],
                        xt[k][0:rows, 512 * g:512 * (g + 1)])
                nc.vector.bn_aggr(ssum[0:rows, 0:2], bnst[0:rows, :])
                nc.vector.tensor_scalar(
                    ssum[0:rows, 1:2], ssum[0:rows, 1:2], 1e-5, None,
                    op0=mybir.AluOpType.add)
                nc.scalar.activation(
                    ssum[0:rows, 2:3], ssum[0:rows, 1:2],
                    mybir.ActivationFunctionType.Sqrt)
                nc.vector.reciprocal(ssum[0:rows, 3:4], ssum[0:rows, 2:3])
                xnk = xn_pool.tile([128, BSH], BF16, tag="xn", name=f"xn_{k}")
                nc.vector.tensor_scalar(
                    xnk[0:rows, :], xt[k][0:rows, 0:BSH],
                    ssum[0:rows, 0:1], ssum[0:rows, 3:4],
                    op0=mybir.AluOpType.subtract, op1=mybir.AluOpType.mult)
                xn.append(xnk)

            # ============ spn add + W1eff fold (vector) ============
            w1e_all = []
            for mt in range(E_MT):
                sl = att_sl(mt)
                spn_sl = aself_t[:, mt * A:(mt + 1) * A]
                nc.vector.tensor_tensor(out=spn_sl, in0=spn_sl, in1=sl,
                                        op=mybir.AluOpType.add)
                w1t = w1full[:, mt * R:(mt + 1) * R]
                w1e_t = w1e_pool.tile([128, R], BF16, tag="w1e",
                                      name=f"w1e_{mt}")
                nc.vector.tensor_tensor(
                    out=w1e_t[:].rearrange("p (a k) -> p a k", k=12),
                    in0=w1t.rearrange("p (a k) -> p a k", k=12),
                    in1=spn_sl.unsqueeze(2).broadcast_to((128, A, 12)),
                    op=mybir.AluOpType.mult)
                w1e_all.append(w1e_t)

            # ============ mask prep (gpsimd, after its DMA issues) ============
            act_f = cpool.tile([A, BSH], F32, tag="actf")
            nc.gpsimd.tensor_copy(act_f[:], act_i[:])
            masks = []
            for c4 in range(NA):
                mask = cpool.tile([A, BSH], F32, tag=f"mask_{c4}",
                                  name=f"mask_{c4}")
                nc.gpsimd.tensor_scalar(
                    mask[:], act_f[:], float(c4), None,
                    op0=mybir.AluOpType.is_equal)
                masks.append(mask)

            # ============ main loop: M1 + pipelined M2 ============
            h1ps = [psh1_pool.tile([128, BSH], F32, tag="h1ps",
                                   name=f"h1ps_{j}") for j in range(3)]
            embs = []

            def emit_m2(mt):
                for j, (c0, c1) in enumerate(R_SPLIT):
                    nc.tensor.matmul(h1ps[j][0:c1 - c0, :],
                                     w1e_all[mt][:, c0:c1], embs[mt][:],
                                     start=(mt == 0), stop=False)

            for mt in range(E_MT):
                ps_mt = ps_pool.tile([128, BSH], F32, tag="ps",
                                     name=f"psm_{mt}")
                for k in range(7):
                    rows = F_T[k]
                    lhsT = wfull[0:rows,
                                 k * E + mt * 128:k * E + (mt + 1) * 128]
                    nc.tensor.matmul(ps_mt[:], lhsT, xn[k][0:rows, :],
                                     start=(k == 0), stop=(k == 6))
                emb = emb_pool.tile([128, BSH], BF16, tag="emb",
                                    name=f"emb_{mt}")
                nc.scalar.activation(emb[:], ps_mt[:],
                                     mybir.ActivationFunctionType.Lrelu,
                                     alpha=0.01)
                embs.append(emb)
            # M2 as one long run AFTER the whole M1 stream: per-mt
            # interleave cost ~570ns/mt in LDW switches + semaphores
            # (stream measured 177ns/matmul vs 120ns for pure runs).
            # All 25 emb tiles are kept resident in SBUF.
            for mt in range(E_MT):
                emit_m2(mt)

            # finish M2: bias row, then leaky
            h1 = []
            for j, (c0, c1) in enumerate(R_SPLIT):
                w = c1 - c0
                nc.tensor.matmul(h1ps[j][0:w, :], b1_t[:, c0:c1], ones_t[:],
                                 start=False, stop=True)
                t = h_pool.tile([128, BSH], BF16, tag=f"h1_{j}",
                                name=f"h1_{j}")
                nc.scalar.activation(t[0:w, :], h1ps[j][0:w, :],
                                     mybir.ActivationFunctionType.Lrelu,
                                     alpha=0.01)
                h1.append(t)

            # M3: h2 = leaky(BD2^T @ h1 + b2)
            h2 = []
            for j, (c0, c1) in enumerate(R_SPLIT):
                w = c1 - c0
                ps3 = ps_pool.tile([128, BSH], F32, tag="ps", name=f"ps3_{j}")
                for k3, (k0, k1) in enumerate(R_SPLIT):
                    nc.tensor.matmul(ps3[0:w, :], bd2_t[k3][:, c0:c1],
                                     h1[k3][0:k1 - k0, :],
                                     start=(k3 == 0), stop=False)
                nc.tensor.matmul(ps3[0:w, :], b2_t[:, c0:c1], ones_t[:],
                                 start=False, stop=True)
                t = h_pool.tile([128, BSH], BF16, tag=f"h2_{j}",
                                name=f"h2_{j}")
                nc.scalar.activation(t[0:w, :], ps3[0:w, :],
                                     mybir.ActivationFunctionType.Lrelu,
                                     alpha=0.01)
                h2.append(t)

            # M4: all_q^T (rows = c*32+a) = BD3^T @ h2 + b3
            ps_q = ps_pool.tile([128, BSH], F32, tag="ps", name="psq")
            for k4, (k0, k1) in enumerate(R_SPLIT):
                nc.tensor.matmul(ps_q[:], bd3_t[k4][:, :],
                                 h2[k4][0:k1 - k0, :],
                                 start=(k4 == 0), stop=False)
            nc.tensor.matmul(ps_q[:], b3_t[:], ones_t[:], start=False,
                             stop=True)

            # gather: q[a, b] = all_q[c(a,b)*32+a, b] via one-hot masks
            qs = []
            for c4 in range(NA):
                qc = cpool.tile([A, BSH], F32, tag=f"qc_{c4}",
                                name=f"qc_{c4}")
                nc.vector.tensor_tensor(
                    out=qc[:], in0=ps_q[c4 * 32:c4 * 32 + A, :],
                    in1=masks[c4][:], op=mybir.AluOpType.mult)
                qs.append(qc)
            nc.vector.tensor_tensor(out=qs[0][:], in0=qs[0][:], in1=qs[1][:],
                                    op=mybir.AluOpType.add)
            nc.vector.tensor_tensor(out=qs[2][:], in0=qs[2][:], in1=qs[3][:],
                                    op=mybir.AluOpType.add)
            nc.vector.tensor_tensor(out=qs[0][:], in0=qs[0][:], in1=qs[2][:],
                                    op=mybir.AluOpType.add)
            nc.sync.dma_start(out_d.ap(), qs[0][:])

    nc.compile()
    return nc


def _host_prep(inputs):
    states = np.asarray(inputs["states"], dtype=np.float32)
    ehh_w = np.asarray(inputs["ehh_w"], dtype=np.float32)
    anova = np.asarray(inputs["anova"], dtype=np.float32)
    w1 = np.asarray(inputs["w1"], dtype=np.float32)
    b1 = np.asarray(inputs["b1"], dtype=np.float32)
    w2 = np.asarray(inputs["w2"], dtype=np.float32)
    b2 = np.asarray(inputs["b2"], dtype=np.float32)
    w3 = np.asarray(inputs["w3"], dtype=np.float32)
    b3 = np.asarray(inputs["b3"], dtype=np.float32)
    actions = np.asarray(inputs["actions"], dtype=np.int32)
    adj = np.asarray(inputs["adj"], dtype=np.int64)

    sT = np.ascontiguousarray(
        states.transpose(0, 2, 1).reshape(F, B)).astype(BF)
    w1T = w1.transpose(1, 0, 2).reshape(E, R)
    # partition-major rearranges (exact SBUF layout -> simple 2D DMAs)
    w1R = np.ascontiguousarray(
        w1T.reshape(E_MT, 128, R).transpose(1, 0, 2).reshape(128, E_MT * R)
    ).astype(BF)
    aselfR = np.ascontiguousarray(
        anova[:E].reshape(E_MT, 128, A).transpose(1, 0, 2)
        .reshape(128, E_MT * A)).astype(BF)

    # adjacency scatter -> winning source row per target (last write wins,
    # col-3 scatter applied after col-1 scatter)
    src = np.full(E, -1, dtype=np.int64)
    for e in range(adj.shape[0]):
        src[adj[e, 1]] = adj[e, 0]
    for e in range(adj.shape[0]):
        src[adj[e, 3]] = adj[e, 0]
    ssel = np.zeros((INTER, E), dtype=np.uint8)
    hit = np.nonzero(src >= 0)[0]
    ssel[src[hit], hit] = 1

    bd2 = np.zeros((R, R), dtype=np.float32)
    bd3 = np.zeros((R, 128), dtype=np.float32)
    b3r = np.zeros((1, 128), dtype=np.float32)
    for a in range(A):
        bd2[12 * a:12 * a + 12, 12 * a:12 * a + 12] = w2[a]
        for c in range(NA):
            bd3[12 * a:12 * a + 12, c * 32 + a] = w3[a, :, c]
            b3r[0, c * 32 + a] = b3[a, c]

    common = {
        "ehh_w": np.ascontiguousarray(ehh_w).astype(BF),
        "w1R": w1R,
        "ssel": ssel,
        "aselfR": aselfR,
        "bi": np.ascontiguousarray(anova[E:]).astype(BF),
        "bd2": bd2.astype(BF),
        "bd3": bd3.astype(BF),
        "b1r": b1.reshape(1, R).astype(BF),
        "b2r": b2.reshape(1, R).astype(BF),
        "b3r": b3r.astype(BF),
        "ones": np.ones((1, BSH), dtype=BF),
    }
    in_maps = []
    for c in range(N_CORES):
        m = dict(common)
        m["sT"] = np.ascontiguousarray(np.roll(sT, -BSH * c, axis=1))
        m["act"] = np.ascontiguousarray(actions[:, BSH * c:BSH * (c + 1)])
        in_maps.append(m)
    return in_maps


def kernel(**inputs):
    global LAST_EXEC_NS
    if "nc" not in _CACHE:
        _CACHE["nc"] = _build_program()
    nc = _CACHE["nc"]
    in_maps = _host_prep(inputs)
    kwargs = {}
    if TRACE:
        kwargs["trace"] = True
    res = bass_utils.run_bass_kernel_spmd(
        nc, in_maps, core_ids=list(range(N_CORES)), **kwargs)
    LAST_EXEC_NS = res.exec_time_ns
    q = np.empty((A, B), dtype=np.float32)
    for c in range(N_CORES):
        q[:, BSH * c:BSH * (c + 1)] = res.results[c]["out"]
    return q
